# revision 53
# baseline (speedup 1.0000x reference)
"""TRN2 Bass kernel for GQA attention (nn_Attention_19533511262498).

Tensor-parallel over heads across 8 NeuronCores: core c owns q-heads
[4c, 4c+4) and kv-head c (wq/wk/wv sharded on the head dim, wo on its
input dim). Each core computes a partial [S, DIM] output (bf16); the
host sums the 8 partials.

Phase 1 (QKV projections) runs in bf16 with 512-wide moving operands:
same PE rate as f32r but half the instruction count and half the DMA
bytes. RoPE is applied with the even/odd weight-row permutation trick
(rotation = elementwise ops + a half-tile partition swap via
SBUF-to-SBUF DMA). q stays resident in SBUF; V is transposed on-chip
per s-block so phase 2 can start immediately.

Phase 2 (attention + output projection) runs in f32r. Causal structure
is exploited at 128-column granularity: for the diagonal kv chunk t of
a 512-wide q block only columns [128t, 512) are computed (scores, exp,
sums, pv), and the 128-wide triangular block is masked by multiplying
exp outputs with a 0/1 triangle, so no mask matmuls are needed.
Softmax skips max-subtraction (scores bounded by construction); the
1/l normalization is applied to pv right before the wo projection and
lags one head behind. The wo projection of q-block b is rationed into
q-block b+1's score loops a few matmuls per chunk so the PE never
waits on the exp->pv dependency chain.
"""

import ml_dtypes
import numpy as np

import concourse.bacc as bacc
import concourse.tile as tile
from concourse import mybir
from concourse.bass import ts, ds
from concourse import bass_isa
from concourse.bass_utils import run_bass_kernel_spmd

F32 = mybir.dt.float32
F32R = mybir.dt.float32r
BF16 = mybir.dt.bfloat16

# problem geometry (hardcoded per contest rules)
S = 2048
DIM = 4096
HD = 128
N_HEADS = 32
N_KV = 8
NCORES = 8
HPC = N_HEADS // NCORES       # 4 q heads per core
FEAT = HPC * HD               # 512 per-core attention feature width

SBW = 512                     # QKV projection s-block width
NSB = S // SBW                # 4
KCH = DIM // 128              # 32 contraction chunks
KQ = 8                        # contraction chunks per x quarter-tile
NXQ = KCH // KQ               # 4
QBW = 512                     # attention q-block width
NQB = S // QBW                # 4
NSC = S // 128                # 16 kv chunks
OBW = 512                     # output-dim block width
NOB = DIM // OBW              # 8

_CACHE = {}


def _build():
    nc = bacc.Bacc("TRN2", target_bir_lowering=False, debug=False,
                   num_devices=NCORES)

    xT = nc.dram_tensor("xT", [NSB, NXQ, 128, KQ, SBW], BF16,
                        kind="ExternalInput").ap()
    wqT = nc.dram_tensor("wqT", [HPC, NXQ, 128, KQ, HD], BF16,
                         kind="ExternalInput").ap()
    wkT = nc.dram_tensor("wkT", [NXQ, 128, KQ, HD], BF16,
                         kind="ExternalInput").ap()
    wvT = nc.dram_tensor("wvT", [NXQ, 128, KQ, HD], BF16,
                         kind="ExternalInput").ap()
    woT = nc.dram_tensor("woT", [HPC, 128, DIM], BF16,
                         kind="ExternalInput").ap()
    cos2 = nc.dram_tensor("cos2", [128, S], F32, kind="ExternalInput").ap()
    sin2 = nc.dram_tensor("sin2", [128, S], F32, kind="ExternalInput").ap()
    sgn = nc.dram_tensor("sgn", [128, 1], F32, kind="ExternalInput").ap()
    ident = nc.dram_tensor("ident", [128, 128], F32, kind="ExternalInput").ap()
    tri = nc.dram_tensor("tri", [128, 128], F32, kind="ExternalInput").ap()
    onesd = nc.dram_tensor("onesd", [128, 128], F32R,
                           kind="ExternalInput").ap()
    out_d = nc.dram_tensor("out", [S, DIM], BF16, kind="ExternalOutput").ap()

    with tile.TileContext(nc) as tc:
        with (
            tc.tile_pool(name="res", bufs=1) as res,
            # psum pools live across both phases so the phase boundary carries
            # no bank anti-deps: mmps = qkv accum + V transposes + wo proj,
            # scps = score tiles, pvps = pv accum.
            tc.tile_pool(name="scps", bufs=3, space="PSUM") as scps,
            tc.tile_pool(name="pvps", bufs=2, space="PSUM") as pvps,
            tc.tile_pool(name="mmps", bufs=3, space="PSUM") as mmps,
            tc.tile_pool(name="wo", bufs=1) as wop,
        ):
            kt_t = res.tile([128, S], BF16, tag="kt")
            v_t = res.tile([128, NSC, HD], F32R, tag="v")
            q_res = res.tile([128, HPC, S], BF16, tag="q")
            sgn_t = res.tile([128, 1], F32, tag="sgn")
            id_t = res.tile([128, 128], F32, tag="id")
            tri_t = res.tile([128, 128], F32, tag="tri")
            ones_t = res.tile([128, 128], F32R, tag="ones")
            nc.scalar.dma_start(out=sgn_t, in_=sgn)
            nc.scalar.dma_start(out=id_t, in_=ident)
            nc.scalar.dma_start(out=tri_t, in_=tri)
            nc.scalar.dma_start(out=ones_t, in_=onesd)

            # ---------------- Phase 1: QKV projections + RoPE ----------------
            with (
                tc.tile_pool(name="wq", bufs=1) as wqp,
                tc.tile_pool(name="wkv", bufs=1) as wkvp,
                tc.tile_pool(name="xt", bufs=6) as xtp,
                tc.tile_pool(name="trig", bufs=2) as trigp,
                tc.tile_pool(name="rope", bufs=4) as ropep,
                tc.tile_pool(name="vt", bufs=2) as vtp,
            ):
                # sync-FIFO order: first wk quarter + first x quarter gate the
                # first matmul; everything else streams behind them.
                wk_qs = [wkvp.tile([128, KQ, HD], BF16, tag=f"wk{i}",
                                   name=f"wk{i}")
                         for i in range(NXQ)]
                nc.sync.dma_start(out=wk_qs[0], in_=wkT[0])

                def load_xq(sb):
                    qs = []
                    for i in range(NXQ):
                        a = xtp.tile([128, KQ, SBW], BF16, tag="xt",
                                     name=f"xq{sb}_{i}")
                        nc.sync.dma_start(out=a, in_=xT[sb, i])
                        qs.append(a)
                    return qs

                xq_tiles = {0: load_xq(0)}
                for i in range(1, NXQ):
                    nc.sync.dma_start(out=wk_qs[i], in_=wkT[i])
                # per-head wq as quarter tiles; head 0's quarters load first
                # so its matmuls start before the rest of the weights land
                wq_hs = [[wqp.tile([128, KQ, HD], BF16, tag=f"wq{h}_{i}",
                                   name=f"wq{h}_{i}")
                          for i in range(NXQ)]
                         for h in range(HPC)]
                for i in range(NXQ):
                    nc.sync.dma_start(out=wq_hs[0][i], in_=wqT[0, i])
                wv_qs = [wkvp.tile([128, KQ, HD], BF16, tag=f"wv{i}",
                                   name=f"wv{i}")
                        for i in range(NXQ)]
                for i in range(NXQ):
                    nc.sync.dma_start(out=wv_qs[i], in_=wvT[i])
                for h in range(1, HPC):
                    for i in range(NXQ):
                        nc.sync.dma_start(out=wq_hs[h][i], in_=wqT[h, i])

                wo_hs = [wop.tile([128, DIM], BF16, tag=f"wo{h}",
                                  name=f"wo{h}")
                         for h in range(HPC)]
                for sb in range(NSB):
                    if sb not in xq_tiles:
                        xq_tiles[sb] = load_xq(sb)
                    if sb + 1 < NSB:
                        xq_tiles[sb + 1] = load_xq(sb + 1)
                    if sb in (1, 2):
                        # wo preload in two 4MB halves so neither burst backs
                        # up the sync queue for long
                        for h in (0, 1) if sb == 1 else (2, 3):
                            nc.sync.dma_start(out=wo_hs[h], in_=woT[h])
                    xq = xq_tiles[sb]
                    c_sl = trigp.tile([128, SBW], F32, tag="cos")
                    nc.scalar.dma_start(out=c_sl, in_=cos2[:, ts(sb, SBW)])
                    s_sl = trigp.tile([128, SBW], F32, tag="sin")
                    nc.scalar.dma_start(out=s_sl, in_=sin2[:, ts(sb, SBW)])
                    # k first, then (for sb 0) q0 before v so the first
                    # s-block follows DMA arrival order: wk, x, wq0, wv.
                    obs = ([HPC, 0, HPC + 1, 1, 2, 3] if sb == 0
                           else [HPC, HPC + 1, 0, 1, 2, 3])
                    for ob in obs:
                        ps = mmps.tile([128, SBW], F32, tag="mm")
                        for k in range(KCH):
                            if ob < HPC:
                                lhs = wq_hs[ob][k // KQ][:, k % KQ, :]
                            elif ob == HPC:
                                lhs = wk_qs[k // KQ][:, k % KQ, :]
                            else:
                                lhs = wv_qs[k // KQ][:, k % KQ, :]
                            rhs = xq[k // KQ][:, k % KQ, :]
                            nc.tensor.matmul(ps, lhs, rhs,
                                             start=(k == 0), stop=(k == KCH - 1))
                        if ob <= HPC:
                            # RoPE: rot = (swap_halves(x*sin) * sgn) + x*cos
                            m1 = ropep.tile([128, SBW], F32, tag="m1")
                            m2 = ropep.tile([128, SBW], F32, tag="m2")
                            w = ropep.tile([128, SBW], F32, tag="w")
                            nc.vector.tensor_mul(m1, ps, c_sl)
                            nc.vector.tensor_mul(m2, ps, s_sl)
                            # swaps on the scalar queue: keeps them clear of
                            # the x/wo bulk traffic on the sync queue
                            nc.scalar.dma_start(out=w[0:64], in_=m2[64:128])
                            nc.scalar.dma_start(out=w[64:128], in_=m2[0:64])
                            if ob < HPC:
                                dst = q_res[:, ob, ts(sb, SBW)]
                            else:
                                dst = kt_t[:, ts(sb, SBW)]
                            nc.vector.scalar_tensor_tensor(
                                dst, w, sgn_t, m1,
                                op0=mybir.AluOpType.mult, op1=mybir.AluOpType.add)
                        else:
                            vt_sb = vtp.tile([128, SBW], F32, tag="vt")
                            nc.vector.tensor_copy(vt_sb, ps)
                            # on-chip V transpose for this s-block's 4 chunks
                            for j in range(SBW // 128):
                                sc = sb * (SBW // 128) + j
                                tr_ps = mmps.tile([128, SBW], F32, tag="mm",
                                                  name=f"tr{sc}")
                                nc.tensor.transpose(
                                    tr_ps[:, 0:HD], vt_sb[:, ts(j, 128)], id_t)
                                nc.vector.tensor_copy(v_t[:, sc, :],
                                                      tr_ps[:, 0:HD])

            # ---------------- Phase 2: attention + output projection --------
            with (
                tc.tile_pool(name="exp", bufs=6) as expp,
                tc.tile_pool(name="outT", bufs=8) as outTp,
                tc.tile_pool(name="rc", bufs=4) as rcp,
                tc.tile_pool(name="stage2", bufs=3) as st2p,
                tc.tile_pool(name="sum", bufs=3) as sump,
            ):
                # Normalizers run 1 head behind the score loop; the wo
                # projection of q-block b is rationed into q-block b+1's
                # score loops as PE filler.
                pend = []  # list of (pv_ps, sum_t, outT_t)

                def emit_normalize():
                    # column sums of sum_t via a ones-matmul (replicates the
                    # partition reduction across all 128 output partitions),
                    # then reciprocal + apply in full-width vector ops.
                    pv_ps, sum_t, outT_t = pend.pop(0)
                    bc_ps = scps.tile([128, QBW], F32, tag="sc", name="bc_ps")
                    nc.tensor.matmul(bc_ps, ones_t, sum_t,
                                     start=True, stop=True)
                    rc_c = rcp.tile([128, QBW], F32, tag="rc")
                    nc.vector.reciprocal_approx_fast(out=rc_c, in_=bc_ps)
                    nc.vector.tensor_mul(outT_t, pv_ps, rc_c)

                # proj work queue: closures, emitted a few per score chunk
                proj_q = []
                copy_par = [0]

                def queue_proj(pqb, tiles):
                    for qs in range(QBW // 128):
                        o_st = st2p.tile([128, DIM], BF16, tag="ost")
                        for ob in range(NOB):
                            p_ps = mmps.tile([128, OBW], F32, tag="mm",
                                             name=f"pr{qs}_{ob}")
                            for h in range(HPC):
                                def mm(p_ps=p_ps, h=h, qs=qs, ob=ob,
                                       tiles=tiles):
                                    nc.tensor.matmul(
                                        p_ps, tiles[h][:, ts(qs, 128)],
                                        wo_hs[h][:, ts(ob, OBW)],
                                        start=(h == 0), stop=(h == HPC - 1))
                                proj_q.append(mm)

                            def cp(p_ps=p_ps, o_st=o_st, ob=ob):
                                if copy_par[0] % 2 == 0:
                                    nc.scalar.copy(o_st[:, ts(ob, OBW)], p_ps)
                                else:
                                    nc.vector.tensor_copy(
                                        o_st[:, ts(ob, OBW)], p_ps)
                                copy_par[0] += 1
                            proj_q.append(cp)

                        def dma(o_st=o_st, pqb=pqb, qs=qs):
                            r = ds(pqb * QBW + qs * 128, 128)
                            nc.scalar.dma_start(
                                out=out_d[r, 0:DIM // 2],
                                in_=o_st[:, 0:DIM // 2])
                            nc.scalar.dma_start(
                                out=out_d[r, DIM // 2:DIM],
                                in_=o_st[:, DIM // 2:DIM])
                        proj_q.append(dma)

                def drain_proj(k):
                    for _ in range(min(k, len(proj_q))):
                        proj_q.pop(0)()

                for qb in range(NQB):
                    outT_tiles = []
                    # chunks remaining in this qb's score loops, for rationing
                    chunks_left = [HPC * (4 * qb + 4)]

                    def ration():
                        if proj_q and chunks_left[0] > 0:
                            k = -(-len(proj_q) // chunks_left[0])  # ceil
                            drain_proj(k)
                        chunks_left[0] -= 1

                    for h in range(HPC):
                        qv = q_res[:, h, ts(qb, QBW)]
                        pv_ps = pvps.tile([128, QBW], F32, tag="pv")
                        nsc = 4 * qb + 4
                        # f32r so the normalize ones-matmul can consume it
                        sum_t = sump.tile([128, QBW], F32R, tag="sum")
                        sum32 = sum_t.bitcast(F32)
                        lag = []  # (s_ps, sc) pending exp/sum/pv

                        def flush_lag():
                            s_ps, sc = lag.pop(0)
                            diag_t = sc - 4 * qb
                            off = 128 * diag_t if diag_t >= 0 else 0
                            wsl = ds(off, QBW - off)
                            e_t = expp.tile([128, QBW], F32R, tag="exp")
                            nc.scalar.activation(
                                e_t[:, wsl], s_ps[:, wsl],
                                mybir.ActivationFunctionType.Exp)
                            e32 = e_t.bitcast(F32)
                            if diag_t >= 0:
                                # causal triangle inside the 128-wide block;
                                # on gpsimd: keeps the exp->mask->pv chain off
                                # the busier vector queue
                                nc.gpsimd.tensor_mul(
                                    e_t[:, ds(off, 128)], e32[:, ds(off, 128)],
                                    tri_t)
                            if sc == 0:
                                nc.vector.tensor_copy(sum_t, e32)
                            else:
                                nc.vector.tensor_add(
                                    sum_t[:, wsl], sum32[:, wsl], e32[:, wsl])
                            nc.tensor.matmul(
                                pv_ps[:, wsl], v_t[:, sc, :], e_t[:, wsl],
                                start=(sc == 0),
                                stop=(sc == 4 * qb + 3 or diag_t >= 0),
                                skip_group_check=True)

                        depth = 1 if proj_q else 2
                        for sc in range(nsc):
                            diag_t = sc - 4 * qb
                            off = 128 * diag_t if diag_t >= 0 else 0
                            wsl = ds(off, QBW - off)
                            s_ps = scps.tile([128, QBW], F32, tag="sc")
                            nc.tensor.matmul(s_ps[:, wsl],
                                             kt_t[:, ts(sc, 128)], qv[:, wsl],
                                             start=True, stop=True)
                            lag.append((s_ps, sc))
                            ration()
                            if len(lag) > depth:
                                flush_lag()
                        while lag:
                            flush_lag()

                        outT_t = outTp.tile([128, QBW], BF16, tag="outT")
                        outT_tiles.append(outT_t)
                        pend.append((pv_ps, sum_t, outT_t))
                        if len(pend) > 1:
                            emit_normalize()
                    while pend:
                        emit_normalize()
                    drain_proj(len(proj_q))
                    queue_proj(qb, outT_tiles)
                drain_proj(len(proj_q))

    nc.compile()
    return nc


def _host_prep(x, wq, wk, wv, wo, freqs_cos, freqs_sin):
    x = np.asarray(x, np.float32)
    wq = np.asarray(wq, np.float32)
    wk = np.asarray(wk, np.float32)
    wv = np.asarray(wv, np.float32)
    wo = np.asarray(wo, np.float32)
    cos = np.asarray(freqs_cos, np.float32)
    sin = np.asarray(freqs_sin, np.float32)

    scale = 1.0 / np.sqrt(np.float32(HD))
    perm = np.concatenate([np.arange(0, HD, 2), np.arange(1, HD, 2)])
    wq_p = (wq.reshape(N_HEADS, HD, DIM)[:, perm, :]).reshape(DIM, DIM) * scale
    wk_p = (wk.reshape(N_KV, HD, DIM)[:, perm, :]).reshape(N_KV * HD, DIM)

    bf = ml_dtypes.bfloat16
    # x tiled: xT[sb, i, p, k, s] = x[0, sb*SBW+s, (i*KQ+k)*128+p]
    xs = x.reshape(S, DIM)
    xT_tiled = np.ascontiguousarray(
        xs.reshape(NSB, SBW, NXQ, KQ, 128).transpose(0, 2, 4, 3, 1)
    ).astype(bf)

    def wtile(wmat_rows):  # [rows<=128, DIM] -> [128, KCH, rows] bf16
        return np.ascontiguousarray(
            wmat_rows.T.reshape(KCH, 128, wmat_rows.shape[0])
            .transpose(1, 0, 2)).astype(bf)

    def wtile_q(wmat_rows):  # quartered: [NXQ, 128, KQ, rows]
        t = wtile(wmat_rows)  # [128, KCH, rows]
        return np.ascontiguousarray(
            t.reshape(128, NXQ, KQ, t.shape[2]).transpose(1, 0, 2, 3))

    def wtile_q4(wmat_rows):
        return wtile_q(wmat_rows)

    cos2 = np.ascontiguousarray(np.concatenate([cos.T, cos.T], 0))
    sin2 = np.ascontiguousarray(np.concatenate([sin.T, sin.T], 0))
    sgnv = np.concatenate([-np.ones((64, 1), np.float32),
                           np.ones((64, 1), np.float32)])
    identm = np.eye(128, dtype=np.float32)
    trim = (np.arange(128)[:, None] <= np.arange(128)[None, :]
            ).astype(np.float32)
    ones128 = np.ones((128, 128), np.float32)

    in_maps = []
    for c in range(NCORES):
        wq_c = wq_p[c * FEAT:(c + 1) * FEAT]
        wqT_tiled = np.stack([wtile_q(wq_c[h * HD:(h + 1) * HD])
                              for h in range(HPC)])
        woc = wo[:, c * FEAT:(c + 1) * FEAT].T  # [FEAT, DIM]
        wo_tiled = np.ascontiguousarray(woc.reshape(HPC, 128, DIM)).astype(bf)
        in_maps.append({
            "xT": xT_tiled,
            "wqT": wqT_tiled,
            "wkT": wtile_q(wk_p[c * HD:(c + 1) * HD]),
            "wvT": wtile_q(wv[c * HD:(c + 1) * HD]),
            "woT": wo_tiled,
            "cos2": cos2,
            "sin2": sin2,
            "sgn": sgnv,
            "ident": identm,
            "tri": trim,
            "onesd": ones128,
        })
    return in_maps


def kernel(x, wq, wk, wv, wo, freqs_cos, freqs_sin, _trace=False):
    if "nc" not in _CACHE:
        _CACHE["nc"] = _build()
    nc = _CACHE["nc"]
    in_maps = _host_prep(x, wq, wk, wv, wo, freqs_cos, freqs_sin)
    res = run_bass_kernel_spmd(nc, in_maps, core_ids=list(range(NCORES)),
                               trace=_trace)
    _CACHE["last_result"] = res
    total = np.zeros((S, DIM), np.float64)
    for c in range(NCORES):
        total += np.asarray(res.results[c]["out"], np.float64)
    return total.astype(np.float32).reshape(1, S, DIM)


# revision 54
# speedup vs baseline: 1.0416x; 1.0416x over previous
"""TRN2 Bass kernel for GQA attention (nn_Attention_19533511262498).

Tensor-parallel over heads across 8 NeuronCores: core c owns q-heads
[4c, 4c+4) and kv-head c (wq/wk/wv sharded on the head dim, wo on its
input dim). Each core computes a partial [S, DIM] output (bf16); the
host sums the 8 partials.

Phase 1 (QKV projections) runs in bf16 with 512-wide moving operands:
same PE rate as f32r but half the instruction count and half the DMA
bytes. RoPE is applied with the even/odd weight-row permutation trick
(rotation = elementwise ops + a half-tile partition swap via
SBUF-to-SBUF DMA). q stays resident in SBUF; V is transposed on-chip
per s-block so phase 2 can start immediately.

Phase 2 (attention + output projection) runs in f32r. Causal structure
is exploited at 128-column granularity: for the diagonal kv chunk t of
a 512-wide q block only columns [128t, 512) are computed (scores, exp,
sums, pv), and the 128-wide triangular block is masked by multiplying
exp outputs with a 0/1 triangle, so no mask matmuls are needed.
Softmax skips max-subtraction (scores bounded by construction); the
1/l normalization is applied to pv right before the wo projection and
lags one head behind. The wo projection of q-block b is rationed into
q-block b+1's score loops a few matmuls per chunk so the PE never
waits on the exp->pv dependency chain.
"""

import ml_dtypes
import numpy as np

import concourse.bacc as bacc
import concourse.tile as tile
from concourse import mybir
from concourse.bass import ts, ds
from concourse import bass_isa
from concourse.bass_utils import run_bass_kernel_spmd

F32 = mybir.dt.float32
F32R = mybir.dt.float32r
BF16 = mybir.dt.bfloat16

# problem geometry (hardcoded per contest rules)
S = 2048
DIM = 4096
HD = 128
N_HEADS = 32
N_KV = 8
NCORES = 8
HPC = N_HEADS // NCORES       # 4 q heads per core
FEAT = HPC * HD               # 512 per-core attention feature width

SBW = 512                     # QKV projection s-block width
NSB = S // SBW                # 4
KCH = DIM // 128              # 32 contraction chunks
KQ = 8                        # contraction chunks per x quarter-tile
NXQ = KCH // KQ               # 4
QBW = 512                     # attention q-block width
NQB = S // QBW                # 4
NSC = S // 128                # 16 kv chunks
OBW = 512                     # output-dim block width
NOB = DIM // OBW              # 8

_CACHE = {}


def _build():
    nc = bacc.Bacc("TRN2", target_bir_lowering=False, debug=False,
                   num_devices=NCORES)

    xT = nc.dram_tensor("xT", [NSB, NXQ, 128, KQ, SBW], BF16,
                        kind="ExternalInput").ap()
    wqT = nc.dram_tensor("wqT", [HPC, NXQ, 128, KQ, HD], BF16,
                         kind="ExternalInput").ap()
    wkT = nc.dram_tensor("wkT", [NXQ, 128, KQ, HD], BF16,
                         kind="ExternalInput").ap()
    wvT = nc.dram_tensor("wvT", [NXQ, 128, KQ, HD], BF16,
                         kind="ExternalInput").ap()
    woT = nc.dram_tensor("woT", [HPC, 128, DIM], BF16,
                         kind="ExternalInput").ap()
    cos2 = nc.dram_tensor("cos2", [128, S], F32, kind="ExternalInput").ap()
    sin2 = nc.dram_tensor("sin2", [128, S], F32, kind="ExternalInput").ap()
    sgn = nc.dram_tensor("sgn", [128, 1], F32, kind="ExternalInput").ap()
    ident = nc.dram_tensor("ident", [128, 128], F32, kind="ExternalInput").ap()
    tri = nc.dram_tensor("tri", [128, 128], F32, kind="ExternalInput").ap()
    onesd = nc.dram_tensor("onesd", [128, 128], F32R,
                           kind="ExternalInput").ap()
    out_d = nc.dram_tensor("out", [S, DIM], BF16, kind="ExternalOutput").ap()

    with tile.TileContext(nc) as tc:
        with (
            tc.tile_pool(name="res", bufs=1) as res,
            # psum pools live across both phases so the phase boundary carries
            # no bank anti-deps: mmps = qkv accum + V transposes + wo proj,
            # scps = score tiles, pvps = pv accum.
            tc.tile_pool(name="scps", bufs=3, space="PSUM") as scps,
            tc.tile_pool(name="pvps", bufs=2, space="PSUM") as pvps,
            tc.tile_pool(name="mmps", bufs=3, space="PSUM") as mmps,
            tc.tile_pool(name="wo", bufs=1) as wop,
        ):
            kt_t = res.tile([128, S], BF16, tag="kt")
            v_t = res.tile([128, NSC, HD], F32R, tag="v")
            q_res = res.tile([128, HPC, S], BF16, tag="q")
            sgn_t = res.tile([128, 1], F32, tag="sgn")
            id_t = res.tile([128, 128], F32, tag="id")
            tri_t = res.tile([128, 128], F32, tag="tri")
            ones_t = res.tile([128, 128], F32R, tag="ones")
            nc.scalar.dma_start(out=sgn_t, in_=sgn)
            nc.scalar.dma_start(out=id_t, in_=ident)
            nc.scalar.dma_start(out=tri_t, in_=tri)
            nc.scalar.dma_start(out=ones_t, in_=onesd)

            # ---------------- Phase 1: QKV projections + RoPE ----------------
            with (
                tc.tile_pool(name="wq", bufs=1) as wqp,
                tc.tile_pool(name="wkv", bufs=1) as wkvp,
                tc.tile_pool(name="xt", bufs=6) as xtp,
                tc.tile_pool(name="trig", bufs=2) as trigp,
                tc.tile_pool(name="rope", bufs=4) as ropep,
                tc.tile_pool(name="vt", bufs=2) as vtp,
            ):
                # sync-FIFO order: first wk quarter + first x quarter gate the
                # first matmul; everything else streams behind them.
                wk_qs = [wkvp.tile([128, KQ, HD], BF16, tag=f"wk{i}",
                                   name=f"wk{i}")
                         for i in range(NXQ)]
                nc.sync.dma_start(out=wk_qs[0], in_=wkT[0])

                def load_xq(sb):
                    qs = []
                    for i in range(NXQ):
                        a = xtp.tile([128, KQ, SBW], BF16, tag="xt",
                                     name=f"xq{sb}_{i}")
                        nc.sync.dma_start(out=a, in_=xT[sb, i])
                        qs.append(a)
                    return qs

                xq_tiles = {0: load_xq(0)}
                for i in range(1, NXQ):
                    nc.sync.dma_start(out=wk_qs[i], in_=wkT[i])
                # per-head wq as quarter tiles; head 0's quarters load first
                # so its matmuls start before the rest of the weights land
                wq_hs = [[wqp.tile([128, KQ, HD], BF16, tag=f"wq{h}_{i}",
                                   name=f"wq{h}_{i}")
                          for i in range(NXQ)]
                         for h in range(HPC)]
                for i in range(NXQ):
                    nc.sync.dma_start(out=wq_hs[0][i], in_=wqT[0, i])
                wv_qs = [wkvp.tile([128, KQ, HD], BF16, tag=f"wv{i}",
                                   name=f"wv{i}")
                        for i in range(NXQ)]
                for i in range(NXQ):
                    nc.sync.dma_start(out=wv_qs[i], in_=wvT[i])
                for h in range(1, HPC):
                    for i in range(NXQ):
                        nc.sync.dma_start(out=wq_hs[h][i], in_=wqT[h, i])

                wo_hs = [wop.tile([128, DIM], BF16, tag=f"wo{h}",
                                  name=f"wo{h}")
                         for h in range(HPC)]
                for sb in range(NSB):
                    if sb not in xq_tiles:
                        xq_tiles[sb] = load_xq(sb)
                    if sb + 1 < NSB:
                        xq_tiles[sb + 1] = load_xq(sb + 1)
                    if sb in (1, 2):
                        # wo preload in two 4MB halves so neither burst backs
                        # up the sync queue for long
                        for h in (0, 1) if sb == 1 else (2, 3):
                            nc.sync.dma_start(out=wo_hs[h], in_=woT[h])
                    xq = xq_tiles[sb]
                    c_sl = trigp.tile([128, SBW], F32, tag="cos")
                    nc.scalar.dma_start(out=c_sl, in_=cos2[:, ts(sb, SBW)])
                    s_sl = trigp.tile([128, SBW], F32, tag="sin")
                    nc.scalar.dma_start(out=s_sl, in_=sin2[:, ts(sb, SBW)])
                    # k first, then (for sb 0) q0 before v so the first
                    # s-block follows DMA arrival order: wk, x, wq0, wv.
                    obs = ([HPC, 0, HPC + 1, 1, 2, 3] if sb == 0
                           else [HPC, HPC + 1, 0, 1, 2, 3])
                    for ob in obs:
                        ps = mmps.tile([128, SBW], F32, tag="mm")
                        for k in range(KCH):
                            if ob < HPC:
                                lhs = wq_hs[ob][k // KQ][:, k % KQ, :]
                            elif ob == HPC:
                                lhs = wk_qs[k // KQ][:, k % KQ, :]
                            else:
                                lhs = wv_qs[k // KQ][:, k % KQ, :]
                            rhs = xq[k // KQ][:, k % KQ, :]
                            nc.tensor.matmul(ps, lhs, rhs,
                                             start=(k == 0), stop=(k == KCH - 1))
                        if ob <= HPC:
                            # RoPE: rot = (swap_halves(x*sin) * sgn) + x*cos
                            m1 = ropep.tile([128, SBW], F32, tag="m1")
                            m2 = ropep.tile([128, SBW], F32, tag="m2")
                            w = ropep.tile([128, SBW], F32, tag="w")
                            nc.vector.tensor_mul(m1, ps, c_sl)
                            nc.vector.tensor_mul(m2, ps, s_sl)
                            nc.sync.dma_start(out=w[0:64], in_=m2[64:128])
                            nc.sync.dma_start(out=w[64:128], in_=m2[0:64])
                            if ob < HPC:
                                dst = q_res[:, ob, ts(sb, SBW)]
                            else:
                                dst = kt_t[:, ts(sb, SBW)]
                            nc.vector.scalar_tensor_tensor(
                                dst, w, sgn_t, m1,
                                op0=mybir.AluOpType.mult, op1=mybir.AluOpType.add)
                        else:
                            vt_sb = vtp.tile([128, SBW], F32, tag="vt")
                            nc.vector.tensor_copy(vt_sb, ps)
                            # on-chip V transpose for this s-block's 4 chunks
                            for j in range(SBW // 128):
                                sc = sb * (SBW // 128) + j
                                tr_ps = mmps.tile([128, SBW], F32, tag="mm",
                                                  name=f"tr{sc}")
                                nc.tensor.transpose(
                                    tr_ps[:, 0:HD], vt_sb[:, ts(j, 128)], id_t)
                                nc.vector.tensor_copy(v_t[:, sc, :],
                                                      tr_ps[:, 0:HD])

            # ---------------- Phase 2: attention + output projection --------
            with (
                tc.tile_pool(name="exp", bufs=6) as expp,
                tc.tile_pool(name="outT", bufs=8) as outTp,
                tc.tile_pool(name="rc", bufs=4) as rcp,
                tc.tile_pool(name="stage2", bufs=3) as st2p,
                tc.tile_pool(name="sum", bufs=3) as sump,
            ):
                # Normalizers run 1 head behind the score loop; the wo
                # projection of q-block b is rationed into q-block b+1's
                # score loops as PE filler.
                pend = []  # list of (pv_ps, sum_t, outT_t)

                def emit_normalize():
                    # column sums of sum_t via a ones-matmul (replicates the
                    # partition reduction across all 128 output partitions),
                    # then reciprocal + apply in full-width vector ops.
                    pv_ps, sum_t, outT_t = pend.pop(0)
                    bc_ps = scps.tile([128, QBW], F32, tag="sc", name="bc_ps")
                    nc.tensor.matmul(bc_ps, ones_t, sum_t,
                                     start=True, stop=True)
                    rc_c = rcp.tile([128, QBW], F32, tag="rc")
                    nc.vector.reciprocal_approx_fast(out=rc_c, in_=bc_ps)
                    nc.vector.tensor_mul(outT_t, pv_ps, rc_c)

                # proj work queue: closures, emitted a few per score chunk
                proj_q = []
                copy_par = [0]

                def queue_proj(pqb, tiles):
                    for qs in range(QBW // 128):
                        o_st = st2p.tile([128, DIM], BF16, tag="ost")
                        for ob in range(NOB):
                            p_ps = mmps.tile([128, OBW], F32, tag="mm",
                                             name=f"pr{qs}_{ob}")
                            for h in range(HPC):
                                def mm(p_ps=p_ps, h=h, qs=qs, ob=ob,
                                       tiles=tiles):
                                    nc.tensor.matmul(
                                        p_ps, tiles[h][:, ts(qs, 128)],
                                        wo_hs[h][:, ts(ob, OBW)],
                                        start=(h == 0), stop=(h == HPC - 1))
                                proj_q.append(mm)

                            def cp(p_ps=p_ps, o_st=o_st, ob=ob):
                                if copy_par[0] % 2 == 0:
                                    nc.scalar.copy(o_st[:, ts(ob, OBW)], p_ps)
                                else:
                                    nc.vector.tensor_copy(
                                        o_st[:, ts(ob, OBW)], p_ps)
                                copy_par[0] += 1
                            proj_q.append(cp)

                        def dma(o_st=o_st, pqb=pqb, qs=qs):
                            r = ds(pqb * QBW + qs * 128, 128)
                            nc.scalar.dma_start(
                                out=out_d[r, 0:DIM // 2],
                                in_=o_st[:, 0:DIM // 2])
                            nc.scalar.dma_start(
                                out=out_d[r, DIM // 2:DIM],
                                in_=o_st[:, DIM // 2:DIM])
                        proj_q.append(dma)

                def drain_proj(k):
                    for _ in range(min(k, len(proj_q))):
                        proj_q.pop(0)()

                for qb in range(NQB):
                    outT_tiles = []
                    # chunks remaining in this qb's score loops, for rationing
                    chunks_left = [HPC * (4 * qb + 4)]

                    def ration():
                        if proj_q and chunks_left[0] > 0:
                            k = -(-len(proj_q) // chunks_left[0])  # ceil
                            drain_proj(k)
                        chunks_left[0] -= 1

                    for h in range(HPC):
                        qv = q_res[:, h, ts(qb, QBW)]
                        pv_ps = pvps.tile([128, QBW], F32, tag="pv")
                        nsc = 4 * qb + 4
                        # f32r so the normalize ones-matmul can consume it
                        sum_t = sump.tile([128, QBW], F32R, tag="sum")
                        sum32 = sum_t.bitcast(F32)
                        lag = []  # (s_ps, sc) pending exp/sum/pv

                        def flush_lag():
                            s_ps, sc = lag.pop(0)
                            diag_t = sc - 4 * qb
                            off = 128 * diag_t if diag_t >= 0 else 0
                            wsl = ds(off, QBW - off)
                            e_t = expp.tile([128, QBW], F32R, tag="exp")
                            nc.scalar.activation(
                                e_t[:, wsl], s_ps[:, wsl],
                                mybir.ActivationFunctionType.Exp)
                            e32 = e_t.bitcast(F32)
                            if diag_t >= 0:
                                # causal triangle inside the 128-wide block;
                                # on gpsimd: keeps the exp->mask->pv chain off
                                # the busier vector queue
                                nc.gpsimd.tensor_mul(
                                    e_t[:, ds(off, 128)], e32[:, ds(off, 128)],
                                    tri_t)
                            if sc == 0:
                                nc.vector.tensor_copy(sum_t, e32)
                            else:
                                nc.vector.tensor_add(
                                    sum_t[:, wsl], sum32[:, wsl], e32[:, wsl])
                            nc.tensor.matmul(
                                pv_ps[:, wsl], v_t[:, sc, :], e_t[:, wsl],
                                start=(sc == 0),
                                stop=(sc == 4 * qb + 3 or diag_t >= 0),
                                skip_group_check=True)

                        depth = 1 if proj_q else 2
                        for sc in range(nsc):
                            diag_t = sc - 4 * qb
                            off = 128 * diag_t if diag_t >= 0 else 0
                            wsl = ds(off, QBW - off)
                            s_ps = scps.tile([128, QBW], F32, tag="sc")
                            nc.tensor.matmul(s_ps[:, wsl],
                                             kt_t[:, ts(sc, 128)], qv[:, wsl],
                                             start=True, stop=True)
                            lag.append((s_ps, sc))
                            ration()
                            if len(lag) > depth:
                                flush_lag()
                        while lag:
                            flush_lag()

                        outT_t = outTp.tile([128, QBW], BF16, tag="outT")
                        outT_tiles.append(outT_t)
                        pend.append((pv_ps, sum_t, outT_t))
                        if len(pend) > 1:
                            emit_normalize()
                    while pend:
                        emit_normalize()
                    drain_proj(len(proj_q))
                    queue_proj(qb, outT_tiles)
                drain_proj(len(proj_q))

    nc.compile()
    return nc


def _host_prep(x, wq, wk, wv, wo, freqs_cos, freqs_sin):
    x = np.asarray(x, np.float32)
    wq = np.asarray(wq, np.float32)
    wk = np.asarray(wk, np.float32)
    wv = np.asarray(wv, np.float32)
    wo = np.asarray(wo, np.float32)
    cos = np.asarray(freqs_cos, np.float32)
    sin = np.asarray(freqs_sin, np.float32)

    scale = 1.0 / np.sqrt(np.float32(HD))
    perm = np.concatenate([np.arange(0, HD, 2), np.arange(1, HD, 2)])
    wq_p = (wq.reshape(N_HEADS, HD, DIM)[:, perm, :]).reshape(DIM, DIM) * scale
    wk_p = (wk.reshape(N_KV, HD, DIM)[:, perm, :]).reshape(N_KV * HD, DIM)

    bf = ml_dtypes.bfloat16
    # x tiled: xT[sb, i, p, k, s] = x[0, sb*SBW+s, (i*KQ+k)*128+p]
    xs = x.reshape(S, DIM)
    xT_tiled = np.ascontiguousarray(
        xs.reshape(NSB, SBW, NXQ, KQ, 128).transpose(0, 2, 4, 3, 1)
    ).astype(bf)

    def wtile(wmat_rows):  # [rows<=128, DIM] -> [128, KCH, rows] bf16
        return np.ascontiguousarray(
            wmat_rows.T.reshape(KCH, 128, wmat_rows.shape[0])
            .transpose(1, 0, 2)).astype(bf)

    def wtile_q(wmat_rows):  # quartered: [NXQ, 128, KQ, rows]
        t = wtile(wmat_rows)  # [128, KCH, rows]
        return np.ascontiguousarray(
            t.reshape(128, NXQ, KQ, t.shape[2]).transpose(1, 0, 2, 3))

    def wtile_q4(wmat_rows):
        return wtile_q(wmat_rows)

    cos2 = np.ascontiguousarray(np.concatenate([cos.T, cos.T], 0))
    sin2 = np.ascontiguousarray(np.concatenate([sin.T, sin.T], 0))
    sgnv = np.concatenate([-np.ones((64, 1), np.float32),
                           np.ones((64, 1), np.float32)])
    identm = np.eye(128, dtype=np.float32)
    trim = (np.arange(128)[:, None] <= np.arange(128)[None, :]
            ).astype(np.float32)
    ones128 = np.ones((128, 128), np.float32)

    in_maps = []
    for c in range(NCORES):
        wq_c = wq_p[c * FEAT:(c + 1) * FEAT]
        wqT_tiled = np.stack([wtile_q(wq_c[h * HD:(h + 1) * HD])
                              for h in range(HPC)])
        woc = wo[:, c * FEAT:(c + 1) * FEAT].T  # [FEAT, DIM]
        wo_tiled = np.ascontiguousarray(woc.reshape(HPC, 128, DIM)).astype(bf)
        in_maps.append({
            "xT": xT_tiled,
            "wqT": wqT_tiled,
            "wkT": wtile_q(wk_p[c * HD:(c + 1) * HD]),
            "wvT": wtile_q(wv[c * HD:(c + 1) * HD]),
            "woT": wo_tiled,
            "cos2": cos2,
            "sin2": sin2,
            "sgn": sgnv,
            "ident": identm,
            "tri": trim,
            "onesd": ones128,
        })
    return in_maps


def kernel(x, wq, wk, wv, wo, freqs_cos, freqs_sin, _trace=False):
    if "nc" not in _CACHE:
        _CACHE["nc"] = _build()
    nc = _CACHE["nc"]
    in_maps = _host_prep(x, wq, wk, wv, wo, freqs_cos, freqs_sin)
    res = run_bass_kernel_spmd(nc, in_maps, core_ids=list(range(NCORES)),
                               trace=_trace)
    _CACHE["last_result"] = res
    total = np.zeros((S, DIM), np.float64)
    for c in range(NCORES):
        total += np.asarray(res.results[c]["out"], np.float64)
    return total.astype(np.float32).reshape(1, S, DIM)
